# revision 1
# baseline (speedup 1.0000x reference)
"""Trainium2 Bass kernel for nn_CategoricalActivation (8-core data-parallel).

Reference semantics (per element x[s, b, h], column col=(b, h)):
    ss = x / (1 + |x|)                            # softsign
    boundaries b_c = ss[ind[c, col], col]         # 4 sampled rows per column
    counts = #{c : ss > b_c} - 2.5
    cat  = cat_u[col] < 0.1
    ord  = (ord_u[col] < 0.7) & cat
    out  = ord ? 0.0 : (cat ? counts : ss)
(The "randomize_classes" remap is identically zero: counts values
{-2.5..1.5} never equal a class id 0..4, so remapped == 0 at ord cols.)

Design (per core, natural [S, C] layout, C = 2 batches x 1024):
  - bulk softsign on [128, 4096] double-chunk tiles:
    |x| and r = 1/(1+|x|) on the Scalar engine (Abs, then spline
    Reciprocal with the +1 folded into the activation bias; ~1.2e-5 max
    rel err, HW-measured), out = x*r with one DVE tensor_tensor.
  - categorical columns are zeroed in the staged bulk input (softsign(0)=0
    gives the exact 0.0 the ord-case needs and pre-clears count columns);
    a separate transposed copy xT keeps the raw values for the gathers.
  - counts: comparisons run on RAW x values, which is exactly
    order-equivalent to comparing softsign values (fl(softsign) is weakly
    monotone; verified elementwise against the reference in test.py).
    The ~3% categorical-non-ord columns are fetched as contiguous xT rows
    by one indirect DMA, compared against per-partition boundary scalars
    (4 fused tensor_scalar / scalar_tensor_tensor passes on the Vector
    engine), and written back compactly to DRAM.
  - host: shards/stages inputs (including the masked bulk copy and the
    transposed copy), passes the padded categorical column list, and
    merges the compact count columns while unsharding (~0.4% of output).
"""

import numpy as np

S = 2048
B = 16
H = 1024
NCORES = 8
BLOC = B // NCORES         # 2
C = BLOC * H               # 2048 columns per core
P = 128
TCH2 = S // P              # 16 chunks
W = C                      # free elements per tile
KMAX = 96                  # padded compact (cat & ~ord) column slots per core
NC5 = 5

_CACHE = {}


def _split_multi_waits(nc, scr_ap=None, max_waits=1):
    """This container's walrus rejects >1 sync-wait per instruction; hoist
    extra waits onto cheap same-engine carrier instructions inserted just
    before (tiny Memset on the pipelined engines - a Drain there would
    flush the pipe at ~0.4-2.4us - and Drain on the sequencer-only ones)."""
    import concourse.mybir as mybir

    memset_engines = {mybir.EngineType.DVE, mybir.EngineType.Pool}
    n_split = 0
    for f in nc.m.functions:
        for blk in f.blocks:
            insts = blk.instructions
            i = 0
            while i < len(insts):
                ins = insts[i]
                si = ins.sync_info
                if si is not None and len(si.on_wait) > max_waits:
                    waits = list(si.on_wait)
                    keep = waits[-max_waits:]
                    hoist = waits[:-max_waits]
                    for w in hoist:
                        if scr_ap is not None and ins.engine in memset_engines:
                            d = mybir.InstMemset(
                                name=f"I-{nc.next_id()}", mode="Const",
                                ins=[], outs=[scr_ap], constant=0)
                        else:
                            d = mybir.InstDrain(
                                name=f"I-{nc.next_id()}", ins=[], outs=[],
                                bass_is_fusable=False)
                        d.engine = ins.engine
                        d.sync_info = mybir.SyncInfo(on_wait=[w], on_update=[])
                        insts.insert(i, d)
                        i += 1
                        n_split += 1
                    si.on_wait = keep
                    ins.sync_info = si
                i += 1
    return n_split


def _act_unary(nc, out_ap, in_ap, func, bias=0.0):
    """One scalar-engine activation, float-immediate bias (bypasses the
    bass wrapper so Reciprocal is allowed; HW-measured ~1.2e-5 max err)."""
    import concourse.mybir as mybir

    eng = nc.scalar
    ins_ = [
        eng.lower_ap(in_ap),
        mybir.ImmediateValue(dtype=mybir.dt.float32, value=float(bias)),
        mybir.ImmediateValue(dtype=mybir.dt.float32, value=1.0),
        mybir.ImmediateValue(dtype=mybir.dt.float32, value=0.0),
    ]
    return eng.add_instruction(
        mybir.InstActivation(
            name=nc.get_next_instruction_name(),
            func=func,
            ins=ins_,
            outs=[eng.lower_ap(out_ap)],
        )
    )


def _build_program():
    import contextlib

    import concourse.bass as bass
    import concourse.tile as tile
    from concourse import mybir

    A = mybir.AluOpType
    F = mybir.ActivationFunctionType
    f32 = mybir.dt.float32
    i32 = mybir.dt.int32

    nc = bass.Bass()
    x_in = nc.dram_tensor("x", [S, C], f32, kind="ExternalInput")
    xt_in = nc.dram_tensor("xT", [C, S], f32, kind="ExternalInput")
    ind_in = nc.dram_tensor("ind", [4, C], i32, kind="ExternalInput")
    gidx_in = nc.dram_tensor("gidx", [KMAX, 1], i32, kind="ExternalInput")
    out_d = nc.dram_tensor("out", [S, C], f32, kind="ExternalOutput")
    cnt_d = nc.dram_tensor("cnt", [KMAX, S], f32, kind="ExternalOutput")

    # wide views: [128, 4096] per 256-row block (contiguous per partition)
    x_wide = x_in[:, :].rearrange("(t p) c -> t p c", p=P)
    out_wide = out_d[:, :].rearrange("(t p) c -> t p c", p=P)

    with tile.TileContext(nc) as tc:
        with contextlib.ExitStack() as ctx:
            singles = ctx.enter_context(tc.tile_pool(name="singles", bufs=1))
            xp = ctx.enter_context(tc.tile_pool(name="xp", bufs=8))
            up = ctx.enter_context(tc.tile_pool(name="up", bufs=4))

            # ---------- phase 0: tiny metadata ----------
            scr = singles.tile([1, 8], i32, name="scr")
            nc.vector.memset(scr, 0)
            gidx = singles.tile([KMAX, 1], i32)
            nc.sync.dma_start(out=gidx, in_=gidx_in[:, :])

            # boundary values b_c[slot] = xT[gcol_slot, ind[c, gcol_slot]]
            gidx_f = singles.tile([KMAX, 1], f32)
            nc.vector.tensor_copy(out=gidx_f, in_=gidx)
            ind_flat = bass.AP(tensor=ind_in[:, :].tensor, offset=0,
                               ap=[[1, 4 * C], [1, 1]])
            xt_flat = bass.AP(tensor=xt_in[:, :].tensor, offset=0,
                              ap=[[1, S * C], [1, 1]])
            bval4 = singles.tile([KMAX, 4], f32)
            for c in range(4):
                offc = singles.tile([KMAX, 1], i32, tag=f"offc{c}", name="offc")
                nc.vector.tensor_scalar(out=offc, in0=gidx, scalar1=c * C,
                                        scalar2=None, op0=A.add)
                indv = singles.tile([KMAX, 1], i32, tag=f"indv{c}", name="indv")
                nc.gpsimd.indirect_dma_start(
                    out=indv, out_offset=None, in_=ind_flat,
                    in_offset=bass.IndirectOffsetOnAxis(ap=offc[:, :1], axis=0))
                indvf = singles.tile([KMAX, 1], f32, tag=f"indvf{c}", name="indvf")
                nc.vector.tensor_copy(out=indvf, in_=indv)
                # offset into xT: gcol * S + ind  (exact in f32, < 2^23)
                xoff_f = singles.tile([KMAX, 1], f32, tag=f"xoff_f{c}", name="xoff_f")
                nc.vector.scalar_tensor_tensor(out=xoff_f, in0=gidx_f,
                                               scalar=float(S), in1=indvf,
                                               op0=A.mult, op1=A.add)
                xoff = singles.tile([KMAX, 1], i32, tag=f"xoff{c}", name="xoff")
                nc.vector.tensor_copy(out=xoff, in_=xoff_f)
                nc.gpsimd.indirect_dma_start(
                    out=bval4[:, c:c + 1], out_offset=None, in_=xt_flat,
                    in_offset=bass.IndirectOffsetOnAxis(ap=xoff[:, :1], axis=0))

            # ---------- compact columns: gather xT rows early ----------
            xct = singles.tile([KMAX, S], f32)
            nc.gpsimd.indirect_dma_start(
                out=xct[:, :], out_offset=None, in_=xt_in[:, :],
                in_offset=bass.IndirectOffsetOnAxis(ap=gidx[:, :1], axis=0))

            # ---------- bulk softsign: out = x * (1 / (1 + |x|)) ----------
            for t in range(TCH2):
                xt = xp.tile([P, W], f32, tag="xt", name="xt")
                nc.sync.dma_start(out=xt, in_=x_wide[t, :, :])
                absx = up.tile([P, W], f32, tag="absx", name="absx")
                _act_unary(nc, absx[:, :], xt[:, :], F.Abs)
                ract = up.tile([P, W], f32, tag="ract", name="ract")
                _act_unary(nc, ract[:, :], absx[:, :], F.Reciprocal, bias=1.0)
                nc.vector.tensor_tensor(out=xt, in0=xt, in1=ract, op=A.mult)
                nc.scalar.dma_start(out=out_wide[t, :, :], in_=xt)

            # ---------- counts on compacted columns (tail work) ----------
            cnt = singles.tile([KMAX, S], f32)
            nc.vector.tensor_scalar(out=cnt, in0=xct, scalar1=bval4[:, 0:1],
                                    scalar2=-2.5, op0=A.is_gt, op1=A.add)
            for c in range(1, 4):
                nc.vector.scalar_tensor_tensor(out=cnt, in0=xct,
                                               scalar=bval4[:, c:c + 1],
                                               in1=cnt, op0=A.is_gt, op1=A.add)
            nc.scalar.dma_start(out=cnt_d[:, :], in_=cnt)

    _split_multi_waits(nc, scr_ap=nc.vector.lower_ap(scr[0:1, 0:1]))
    return nc


def kernel(x, ind, cat_u, ord_u, perm, num_classes):
    from concourse.bass_utils import run_bass_kernel_spmd

    assert int(num_classes) == NC5
    x = np.ascontiguousarray(x, dtype=np.float32)
    ind = np.ascontiguousarray(ind, dtype=np.int32)
    cat_u = np.asarray(cat_u, dtype=np.float32)
    ord_u = np.asarray(ord_u, dtype=np.float32)
    assert x.shape == (S, B, H) and ind.shape == (4, B, H)

    cat = cat_u < np.float32(0.1)
    catno = cat & ~(ord_u < np.float32(0.7))      # columns that need counts
    in_maps = []
    col_lists = []
    for m in range(NCORES):
        bs = slice(BLOC * m, BLOC * (m + 1))
        xm = np.ascontiguousarray(x[:, bs, :].reshape(S, C))
        xtm = np.ascontiguousarray(xm.T)          # raw values for gathers
        catcols = np.nonzero(cat[bs].reshape(C))[0]
        xm[:, catcols] = 0.0                      # softsign(0) == 0 == ord out
        indm = np.ascontiguousarray(ind[:, bs, :].reshape(4, C))
        cols = np.nonzero(catno[bs].reshape(C))[0].astype(np.int32)
        k = len(cols)
        assert k <= KMAX, f"core {m}: {k} categorical columns exceed KMAX"
        col_lists.append(cols)
        gidx = np.zeros((KMAX, 1), np.int32)
        gidx[:k, 0] = cols
        in_maps.append({"x": xm, "xT": xtm, "ind": indm, "gidx": gidx})

    if "nc" not in _CACHE:
        _CACHE["nc"] = _build_program()
    res = run_bass_kernel_spmd(_CACHE["nc"], in_maps,
                               core_ids=list(range(NCORES)))
    out = np.empty((S, B, H), np.float32)
    for m in range(NCORES):
        om = res.results[m]["out"]                # [S, C]
        cols = col_lists[m]
        if len(cols):
            om = om.copy()
            om[:, cols] = res.results[m]["cnt"][:len(cols)].T
        out[:, BLOC * m:BLOC * (m + 1), :] = om.reshape(S, BLOC, H)
    return out



# revision 4
# speedup vs baseline: 1.5139x; 1.5139x over previous
"""Trainium2 Bass kernel for nn_CategoricalActivation (8-core data-parallel).

Reference semantics (per element x[s, b, h], column col=(b, h)):
    ss = x / (1 + |x|)                            # softsign
    boundaries b_c = ss[ind[c, col], col]         # 4 sampled rows per column
    counts = #{c : ss > b_c} - 2.5
    cat  = cat_u[col] < 0.1
    ord  = (ord_u[col] < 0.7) & cat
    out  = ord ? 0.0 : (cat ? counts : ss)
(The "randomize_classes" remap is identically zero: counts values
{-2.5..1.5} never equal a class id 0..4, so remapped == 0 at ord cols.)

Design (per core; the bulk softsign path is elementwise, so the core's
[S, C] slab is staged flat as [1024, 4096] to give eight fully
contiguous 1 MiB DMA tiles):
  - bulk path runs in bf16 end to end (max rel err ~6e-3, the gate is
    2e-2): half the HBM traffic of the f32 baseline, which was
    DMA-bound.  Per [128, 4096] tile: |x| via one DVE tensor_scalar
    (abs_max with 0, 4x bf16 mode), r = 1/(1+|x|) on the Scalar engine
    (spline Reciprocal, +1 folded into the activation bias), out = x*r
    in-place with one DVE tensor_tensor (2x bf16 mode).
  - categorical columns are zeroed in the staged bulk input
    (softsign(0) = 0 gives the exact 0.0 the ord-case needs and
    pre-clears count columns).
  - counts stay exact: comparisons run on RAW f32 x values, which is
    order-equivalent to comparing softsign values (fl(softsign) is
    weakly monotone; verified elementwise in test.py).  The ~3%
    categorical-non-ord columns and their 4 boundary values per column
    are host-gathered (compacted) into [KMAX, S] f32 / [KMAX, 4] f32,
    compared against per-partition boundary scalars on the DVE (4 fused
    passes), and written back as exact bf16 (counts are multiples of
    0.5, all representable).
  - engine split: loads on SP's HWDGE ring, stores on the gpsimd SWDGE
    ring, Reciprocal on Scalar, everything elementwise on DVE - no
    engine both computes and triggers a DMA it must wait on.
  - host: shards/stages inputs (bf16 cast, masked copy, compaction) and
    merges the compact count columns while unsharding (~3% of output).
"""

import numpy as np

S = 2048
B = 16
H = 1024
NCORES = 8
BLOC = B // NCORES         # 2
C = BLOC * H               # 2048 columns per core
P = 128
RB = 1024                  # staged bulk rows:  [1024, 4096] == [S, C] flat
WB = 4096
NT = RB // P               # 8 bulk tiles of [128, 4096] bf16 (1 MiB each)
KMAX = 96                  # padded compact (cat & ~ord) column slots per core
NC5 = 5

_CACHE = {}


def _split_multi_waits(nc, scr_ap=None, max_waits=1):
    """This container's walrus rejects >1 sync-wait per instruction; hoist
    extra waits onto cheap same-engine carrier instructions inserted just
    before (tiny Memset on the pipelined engines - a Drain there would
    flush the pipe at ~0.4-2.4us - and Drain on the sequencer-only ones)."""
    import concourse.mybir as mybir

    memset_engines = {mybir.EngineType.DVE, mybir.EngineType.Pool}
    n_split = 0
    for f in nc.m.functions:
        for blk in f.blocks:
            insts = blk.instructions
            i = 0
            while i < len(insts):
                ins = insts[i]
                si = ins.sync_info
                if si is not None and len(si.on_wait) > max_waits:
                    waits = list(si.on_wait)
                    keep = waits[-max_waits:]
                    hoist = waits[:-max_waits]
                    for w in hoist:
                        if scr_ap is not None and ins.engine in memset_engines:
                            d = mybir.InstMemset(
                                name=f"I-{nc.next_id()}", mode="Const",
                                ins=[], outs=[scr_ap], constant=0)
                        else:
                            d = mybir.InstDrain(
                                name=f"I-{nc.next_id()}", ins=[], outs=[],
                                bass_is_fusable=False)
                        d.engine = ins.engine
                        d.sync_info = mybir.SyncInfo(on_wait=[w], on_update=[])
                        insts.insert(i, d)
                        i += 1
                        n_split += 1
                    si.on_wait = keep
                    ins.sync_info = si
                i += 1
    return n_split


def _act_unary(nc, out_ap, in_ap, func, bias=0.0):
    """One scalar-engine activation, float-immediate bias (bypasses the
    bass wrapper so Reciprocal is allowed; HW-measured ~1.2e-5 max err)."""
    import concourse.mybir as mybir

    eng = nc.scalar
    ins_ = [
        eng.lower_ap(in_ap),
        mybir.ImmediateValue(dtype=mybir.dt.float32, value=float(bias)),
        mybir.ImmediateValue(dtype=mybir.dt.float32, value=1.0),
        mybir.ImmediateValue(dtype=mybir.dt.float32, value=0.0),
    ]
    return eng.add_instruction(
        mybir.InstActivation(
            name=nc.get_next_instruction_name(),
            func=func,
            ins=ins_,
            outs=[eng.lower_ap(out_ap)],
        )
    )


def _build_program():
    import contextlib

    import concourse.bass as bass
    import concourse.tile as tile
    from concourse import mybir

    A = mybir.AluOpType
    F = mybir.ActivationFunctionType
    f32 = mybir.dt.float32
    bf16 = mybir.dt.bfloat16
    i16 = mybir.dt.int16

    nc = bass.Bass()
    xb_in = nc.dram_tensor("xb", [RB, WB], bf16, kind="ExternalInput")
    xc_in = nc.dram_tensor("xc", [KMAX, S], f32, kind="ExternalInput")
    bv_in = nc.dram_tensor("bv", [KMAX, 4], f32, kind="ExternalInput")
    out_d = nc.dram_tensor("out", [RB, WB], bf16, kind="ExternalOutput")
    cnt_d = nc.dram_tensor("cnt", [KMAX, S], bf16, kind="ExternalOutput")

    x_wide = xb_in[:, :].rearrange("(t p) c -> t p c", p=P)
    out_wide = out_d[:, :].rearrange("(t p) c -> t p c", p=P)

    with tile.TileContext(nc) as tc:
        with contextlib.ExitStack() as ctx:
            singles = ctx.enter_context(tc.tile_pool(name="singles", bufs=1))
            xp = ctx.enter_context(tc.tile_pool(name="xp", bufs=4))
            up = ctx.enter_context(tc.tile_pool(name="up", bufs=3))

            scr = singles.tile([1, 8], mybir.dt.int32, name="scr")
            nc.vector.memset(scr, 0)

            # compact-column inputs (Scalar's HWDGE ring; tiny + early)
            xc = singles.tile([KMAX, S], f32)
            nc.scalar.dma_start(out=xc, in_=xc_in[:, :])
            bv = singles.tile([KMAX, 4], f32)
            nc.scalar.dma_start(out=bv, in_=bv_in[:, :])
            cntf = singles.tile([KMAX, S], f32)
            cntb = singles.tile([KMAX, S], bf16)

            # bulk softsign: out = x * (1 / (1 + |x|)), all bf16.
            # Counts DVE passes are spread between early tiles so they
            # fill DVE gaps while DMA streams.
            for t in range(NT):
                xt = xp.tile([P, WB], bf16, tag="xt", name="xt")
                nc.sync.dma_start(out=xt, in_=x_wide[t, :, :])
                t1 = up.tile([P, WB], bf16, tag="t1", name="t1")
                # |x| = clear the bf16 sign bit (abs_max isn't a valid
                # TensorScalar ISA op; this is exact and keeps 4x mode)
                nc.vector.tensor_scalar(out=t1[:, :].bitcast(i16),
                                        in0=xt[:, :].bitcast(i16),
                                        scalar1=0x7FFF, scalar2=None,
                                        op0=A.bitwise_and)
                r = up.tile([P, WB], bf16, tag="r", name="r")
                _act_unary(nc, r[:, :], t1[:, :], F.Reciprocal, bias=1.0)
                if t == 2:
                    nc.vector.tensor_scalar(out=cntf, in0=xc,
                                            scalar1=bv[:, 0:1], scalar2=-2.5,
                                            op0=A.is_gt, op1=A.add)
                elif t in (3, 4):
                    c = t - 2
                    nc.vector.scalar_tensor_tensor(out=cntf, in0=xc,
                                                   scalar=bv[:, c:c + 1],
                                                   in1=cntf, op0=A.is_gt,
                                                   op1=A.add)
                elif t == 5:
                    nc.vector.scalar_tensor_tensor(out=cntb, in0=xc,
                                                   scalar=bv[:, 3:4],
                                                   in1=cntf, op0=A.is_gt,
                                                   op1=A.add)
                nc.vector.tensor_tensor(out=xt, in0=xt, in1=r, op=A.mult)
                nc.gpsimd.dma_start(out=out_wide[t, :, :], in_=xt)
                if t == 5:
                    nc.scalar.dma_start(out=cnt_d[:, :], in_=cntb)

    _split_multi_waits(nc, scr_ap=nc.vector.lower_ap(scr[0:1, 0:1]))
    return nc


def kernel(x, ind, cat_u, ord_u, perm, num_classes):
    import ml_dtypes
    from concourse.bass_utils import run_bass_kernel_spmd

    assert int(num_classes) == NC5
    x = np.ascontiguousarray(x, dtype=np.float32)
    ind = np.ascontiguousarray(ind, dtype=np.int32)
    cat_u = np.asarray(cat_u, dtype=np.float32)
    ord_u = np.asarray(ord_u, dtype=np.float32)
    assert x.shape == (S, B, H) and ind.shape == (4, B, H)
    bf = ml_dtypes.bfloat16

    cat = cat_u < np.float32(0.1)
    catno = cat & ~(ord_u < np.float32(0.7))      # columns that need counts
    in_maps = []
    col_lists = []
    for m in range(NCORES):
        bs = slice(BLOC * m, BLOC * (m + 1))
        xm = np.ascontiguousarray(x[:, bs, :].reshape(S, C))
        indm = ind[:, bs, :].reshape(4, C)
        cols = np.nonzero(catno[bs].reshape(C))[0]
        k = len(cols)
        assert k <= KMAX, f"core {m}: {k} categorical columns exceed KMAX"
        col_lists.append(cols)
        xc = np.zeros((KMAX, S), np.float32)
        xc[:k] = xm[:, cols].T                    # raw values for exact counts
        bv = np.zeros((KMAX, 4), np.float32)
        bv[:k] = xm[indm[:, cols], cols[None, :]].T
        catcols = np.nonzero(cat[bs].reshape(C))[0]
        xm[:, catcols] = 0.0                      # softsign(0) == 0 == ord out
        xb = xm.reshape(RB, WB).astype(bf)
        in_maps.append({"xb": xb, "xc": xc, "bv": bv})

    if "nc" not in _CACHE:
        _CACHE["nc"] = _build_program()
    res = run_bass_kernel_spmd(_CACHE["nc"], in_maps,
                               core_ids=list(range(NCORES)))
    out = np.empty((S, B, H), np.float32)
    for m in range(NCORES):
        om = np.asarray(res.results[m]["out"]).astype(np.float32)
        om = om.reshape(S, C)
        cols = col_lists[m]
        if len(cols):
            cnt = np.asarray(res.results[m]["cnt"]).astype(np.float32)
            om[:, cols] = cnt[:len(cols)].T
        out[:, BLOC * m:BLOC * (m + 1), :] = om.reshape(S, BLOC, H)
    return out
